# revision 10
# baseline (speedup 1.0000x reference)
"""DTNN layer kernel for Trainium2 (8 NeuronCores).

Math: out[b,i,o] = sum_j sum_h Wfc[o,h] * hx[b,i,h] * hd[b,i,j,h]
with hx = x@Wcf.T + bcf, hd = dist@Wdf.T + bdf.
Since Wfc/Wdf are linear, the j-sum commutes:
    ds[b,i,d]  = sum_j dist[b,i,j,d]                  (memory-bound reduction)
    out[b,i,:] = ((x@Wcf.T + bcf) * (ds@Wdf.T + N*bdf)) @ Wfc.T
So the kernel streams `distance` once (134MB) and does a few 128x128 matmuls.

Sharding: flatten (B,N) -> 1024 i-rows, 128 rows per core; no cross-core comms.

Hardware quirks honored here:
- compute instructions tolerate only ONE sync-wait, so all constants arrive
  via a single dma_start (one sem), biases are folded into the PE matmuls as
  K=1 rank-1 updates (bias_row^T @ ones_row), and DVE ops are arranged to
  have either one cross-engine wait or same-engine deps only.
"""

import numpy as np

import concourse.bass as bass
import concourse.bacc as bacc
import concourse.mybir as mybir
from concourse.tile import TileContext
from concourse.bass_utils import run_bass_kernel_spmd

B, N, D, H = 4, 256, 128, 128
NCORES = 8
ROWS = B * N // NCORES  # 128 i-rows per core
FP = mybir.dt.float32

J_TILE = 32             # j-columns per DMA block (2MB per DMA)
N_BLK = N // J_TILE     # 8 blocks

# packed constant columns: [xT | wcfT | wdfT | wfcT | eye | rows...]
C_XT = 0
C_WCF = 128
C_WDF = 256
C_WFC = 384
C_EYE = 512
C_BCFR = 640   # partition 0: bcf row (1, H)
C_BDFR = 768   # partition 0: bdf row (1, H)
C_ONES = 896   # partition 0: ones row (1, ROWS)
C_N256 = 1024  # partition 0: value-N row (1, ROWS)
C_TOT = 1152


def build_nc():
    nc = bacc.Bacc("TRN2", target_bir_lowering=False)
    dist = nc.declare_dram_parameter("dist", [ROWS, N * D], FP, isOutput=False)
    cst = nc.declare_dram_parameter("cst", [128, C_TOT], FP, isOutput=False)
    out = nc.declare_dram_parameter("out", [ROWS, D], FP, isOutput=True)

    with TileContext(nc) as tc:
        with (
            tc.tile_pool(name="const", bufs=1) as cpool,
            tc.tile_pool(name="dist", bufs=N_BLK) as dpool,
            tc.tile_pool(name="work", bufs=1) as wpool,
            tc.tile_pool(name="psum", bufs=1, space="PSUM") as ppool,
        ):
            cst_t = cpool.tile([128, C_TOT], FP)
            nc.sync.dma_start(out=cst_t[:], in_=cst[:])
            xT_t = cst_t[:, C_XT:C_XT + ROWS]
            wcf_t = cst_t[:, C_WCF:C_WCF + H]
            wdf_t = cst_t[:, C_WDF:C_WDF + H]
            wfc_t = cst_t[:, C_WFC:C_WFC + D]
            ident = cst_t[:, C_EYE:C_EYE + ROWS]
            bcf_row = cst_t[0:1, C_BCFR:C_BCFR + H]
            bdf_row = cst_t[0:1, C_BDFR:C_BDFR + H]
            ones_row = cst_t[0:1, C_ONES:C_ONES + ROWS]
            n256_row = cst_t[0:1, C_N256:C_N256 + ROWS]

            # hx^T = (Wcf^T)^T @ x^T + bcf x ones -> (H, ROWS) in PSUM
            hx_ps = ppool.tile([H, ROWS], FP)
            nc.tensor.matmul(hx_ps[:], wcf_t, xT_t, start=True, stop=False)
            nc.tensor.matmul(hx_ps[:], bcf_row, ones_row, start=False, stop=True)
            hxT = wpool.tile([H, ROWS], FP)
            nc.vector.tensor_copy(hxT[:], hx_ps[:])

            # Streaming j-reduction: ds[i,d] = sum_j dist[i,j,d]
            partials = wpool.tile([ROWS, N_BLK * D], FP)
            for jb in range(N_BLK):
                dt_ = dpool.tile([ROWS, J_TILE * D], FP)
                nc.sync.dma_start(
                    out=dt_[:],
                    in_=dist[:, jb * J_TILE * D:(jb + 1) * J_TILE * D],
                )
                v = dt_[:].rearrange("p (j d) -> p d j", j=J_TILE)
                nc.vector.tensor_reduce(
                    out=partials[:, jb * D:(jb + 1) * D],
                    in_=v,
                    axis=mybir.AxisListType.X,
                    op=mybir.AluOpType.add,
                )
            ds = wpool.tile([ROWS, D], FP)
            nc.vector.tensor_reduce(
                out=ds[:],
                in_=partials[:].rearrange("p (b d) -> p d b", b=N_BLK),
                axis=mybir.AxisListType.X,
                op=mybir.AluOpType.add,
            )

            # ds (i,d) -> dsT (d,i) via PE transpose
            dsT_ps = ppool.tile([D, ROWS], FP)
            nc.tensor.transpose(dsT_ps[:], ds[:], ident)
            dsT = wpool.tile([D, ROWS], FP)
            nc.vector.tensor_copy(dsT[:], dsT_ps[:])

            # hd^T = (Wdf^T)^T @ ds^T + N * bdf x ones -> (H, ROWS)
            hd_ps = ppool.tile([H, ROWS], FP)
            nc.tensor.matmul(hd_ps[:], wdf_t, dsT[:], start=True, stop=False)
            nc.tensor.matmul(hd_ps[:], bdf_row, n256_row, start=False, stop=True)
            hdT = wpool.tile([H, ROWS], FP)
            nc.vector.tensor_copy(hdT[:], hd_ps[:])

            # s^T = hx^T * hd^T (both DVE-resident -> single same-engine dep)
            sT = wpool.tile([H, ROWS], FP)
            nc.vector.tensor_mul(sT[:], hdT[:], hxT[:])

            # out = sT^T @ Wfc^T -> (ROWS, D)
            out_ps = ppool.tile([ROWS, D], FP)
            nc.tensor.matmul(out_ps[:], sT[:], wfc_t, start=True, stop=True)
            out_sb = wpool.tile([ROWS, D], FP)
            nc.vector.tensor_copy(out_sb[:], out_ps[:])
            # SWDGE store: its DMASW sem lane is fresh, so this carries only
            # the DVE wait (HWDGE lane reuse would add a second wait).
            nc.gpsimd.dma_start(out=out[:], in_=out_sb[:])
    nc.compile()
    return nc


_NC_CACHE = None


def _get_nc():
    global _NC_CACHE
    if _NC_CACHE is None:
        _NC_CACHE = build_nc()
    return _NC_CACHE


def _make_in_maps(x, distance, Wcf_w, Wcf_b, Wdf_w, Wdf_b, Wfc_w):
    x = np.ascontiguousarray(np.asarray(x, np.float32))
    distance = np.ascontiguousarray(np.asarray(distance, np.float32))
    x_flat = x.reshape(B * N, D)
    dist_flat = distance.reshape(B * N, N * D)
    wcfT = np.asarray(Wcf_w, np.float32).T
    wdfT = np.asarray(Wdf_w, np.float32).T
    wfcT = np.asarray(Wfc_w, np.float32).T
    bcf = np.asarray(Wcf_b, np.float32)
    bdf = np.asarray(Wdf_b, np.float32)
    in_maps = []
    for c in range(NCORES):
        sl = slice(c * ROWS, (c + 1) * ROWS)
        cstblk = np.zeros((128, C_TOT), np.float32)
        cstblk[:, C_XT:C_XT + ROWS] = x_flat[sl].T
        cstblk[:, C_WCF:C_WCF + H] = wcfT
        cstblk[:, C_WDF:C_WDF + H] = wdfT
        cstblk[:, C_WFC:C_WFC + D] = wfcT
        cstblk[:, C_EYE:C_EYE + ROWS] = np.eye(ROWS, dtype=np.float32)
        cstblk[0, C_BCFR:C_BCFR + H] = bcf
        cstblk[0, C_BDFR:C_BDFR + H] = bdf
        cstblk[0, C_ONES:C_ONES + ROWS] = 1.0
        cstblk[0, C_N256:C_N256 + ROWS] = float(N)
        in_maps.append({
            "dist": np.ascontiguousarray(dist_flat[sl]),
            "cst": cstblk,
        })
    return in_maps


def kernel(x, distance, Wcf_w, Wcf_b, Wdf_w, Wdf_b, Wfc_w):
    in_maps = _make_in_maps(x, distance, Wcf_w, Wcf_b, Wdf_w, Wdf_b, Wfc_w)
    nc = _get_nc()
    res = run_bass_kernel_spmd(nc, in_maps, list(range(NCORES))).results
    out = np.concatenate([res[c]["out"] for c in range(NCORES)], axis=0)
    return out.reshape(B, N, D)


# revision 13
# speedup vs baseline: 1.0555x; 1.0555x over previous
"""DTNN layer kernel for Trainium2 (8 NeuronCores).

Math: out[b,i,o] = sum_j sum_h Wfc[o,h] * hx[b,i,h] * hd[b,i,j,h]
with hx = x@Wcf.T + bcf, hd = dist@Wdf.T + bdf.
Since Wfc/Wdf are linear, the j-sum commutes:
    ds[b,i,d]  = sum_j dist[b,i,j,d]                  (memory-bound reduction)
    out[b,i,:] = ((x@Wcf.T + bcf) * (ds@Wdf.T + N*bdf)) @ Wfc.T
So the kernel streams `distance` once (134MB) and does a few 128x128 matmuls.

Sharding: flatten (B,N) -> 1024 i-rows, 128 rows per core; no cross-core comms.

Hardware quirks honored here:
- compute instructions tolerate only ONE sync-wait, so all constants arrive
  via a single dma_start (one sem), biases are folded into the PE matmuls as
  K=1 rank-1 updates (bias_row^T @ ones_row), and DVE ops are arranged to
  have either one cross-engine wait or same-engine deps only.
"""

import numpy as np

import concourse.bass as bass
import concourse.bacc as bacc
import concourse.mybir as mybir
from concourse.tile import TileContext
from concourse.bass_utils import run_bass_kernel_spmd

B, N, D, H = 4, 256, 128, 128
NCORES = 8
ROWS = B * N // NCORES  # 128 i-rows per core
FP = mybir.dt.float32

J_TILE = 32             # j-columns per DMA block (2MB per DMA)
N_BLK = N // J_TILE     # 8 blocks

# packed constant columns: [xT | wcfT | wdfT | wfcT | eye | rows...]
C_XT = 0
C_WCF = 128
C_WDF = 256
C_WFC = 384
C_EYE = 512
C_BCFR = 640   # partition 0: bcf row (1, H)
C_BDFR = 768   # partition 0: bdf row (1, H)
C_ONES = 896   # partition 0: ones row (1, ROWS)
C_N256 = 1024  # partition 0: value-N row (1, ROWS)
C_TOT = 1152


def build_nc():
    nc = bacc.Bacc("TRN2", target_bir_lowering=False)
    dist = nc.declare_dram_parameter("dist", [ROWS, N * D], FP, isOutput=False)
    cst = nc.declare_dram_parameter("cst", [128, C_TOT], FP, isOutput=False)
    out = nc.declare_dram_parameter("out", [ROWS, D], FP, isOutput=True)

    with TileContext(nc) as tc:
        with (
            tc.tile_pool(name="const", bufs=1) as cpool,
            tc.tile_pool(name="dist", bufs=1) as dpool,
            tc.tile_pool(name="work", bufs=1) as wpool,
            tc.tile_pool(name="psum", bufs=1, space="PSUM") as ppool,
        ):
            cst_t = cpool.tile([128, C_TOT], FP)
            nc.sync.dma_start(out=cst_t[:], in_=cst[:])
            xT_t = cst_t[:, C_XT:C_XT + ROWS]
            wcf_t = cst_t[:, C_WCF:C_WCF + H]
            wdf_t = cst_t[:, C_WDF:C_WDF + H]
            wfc_t = cst_t[:, C_WFC:C_WFC + D]
            ident = cst_t[:, C_EYE:C_EYE + ROWS]
            bcf_row = cst_t[0:1, C_BCFR:C_BCFR + H]
            bdf_row = cst_t[0:1, C_BDFR:C_BDFR + H]
            ones_row = cst_t[0:1, C_ONES:C_ONES + ROWS]
            n256_row = cst_t[0:1, C_N256:C_N256 + ROWS]

            # hx^T = (Wcf^T)^T @ x^T + bcf x ones -> (H, ROWS) in PSUM
            hx_ps = ppool.tile([H, ROWS], FP)
            nc.tensor.matmul(hx_ps[:], wcf_t, xT_t, start=True, stop=False)
            nc.tensor.matmul(hx_ps[:], bcf_row, ones_row, start=False, stop=True)
            hxT = wpool.tile([H, ROWS], FP)
            nc.vector.tensor_copy(hxT[:], hx_ps[:])

            # Streaming j-reduction: ds[i,d] = sum_j dist[i,j,d].
            # All adds are unit-stride (full DVE rate); strided reduces run
            # at ~60% rate and became the critical path.
            W = J_TILE * D
            tiles = []
            for jb in range(N_BLK):
                dt_ = dpool.tile([ROWS, W], FP, tag=f"dist{jb}")
                nc.sync.dma_start(
                    out=dt_[:],
                    in_=dist[:, jb * W:(jb + 1) * W],
                )
                tiles.append(dt_)
            # pairwise tree over the 8 tiles: 7 wide adds
            stride = 1
            while stride < N_BLK:
                for a in range(0, N_BLK, 2 * stride):
                    nc.vector.tensor_add(
                        tiles[a][:], tiles[a][:], tiles[a + stride][:]
                    )
                stride *= 2
            # fold j inside tile 0: halve 4096 -> 128 (j-major layout)
            t0 = tiles[0]
            half = W // 2
            while half >= D:
                nc.vector.tensor_add(
                    t0[:, 0:half], t0[:, 0:half], t0[:, half:2 * half]
                )
                half //= 2
            ds = t0[:, 0:D]

            # ds (i,d) -> dsT (d,i) via PE transpose
            dsT_ps = ppool.tile([D, ROWS], FP)
            nc.tensor.transpose(dsT_ps[:], ds, ident)
            dsT = wpool.tile([D, ROWS], FP)
            nc.vector.tensor_copy(dsT[:], dsT_ps[:])

            # hd^T = (Wdf^T)^T @ ds^T + N * bdf x ones -> (H, ROWS)
            hd_ps = ppool.tile([H, ROWS], FP)
            nc.tensor.matmul(hd_ps[:], wdf_t, dsT[:], start=True, stop=False)
            nc.tensor.matmul(hd_ps[:], bdf_row, n256_row, start=False, stop=True)

            # s^T = hx^T * hd^T (read PSUM directly)
            sT = wpool.tile([H, ROWS], FP)
            nc.vector.tensor_mul(sT[:], hd_ps[:], hxT[:])

            # out = sT^T @ Wfc^T -> (ROWS, D)
            out_ps = ppool.tile([ROWS, D], FP)
            nc.tensor.matmul(out_ps[:], sT[:], wfc_t, start=True, stop=True)
            out_sb = wpool.tile([ROWS, D], FP)
            nc.vector.tensor_copy(out_sb[:], out_ps[:])
            # SWDGE store: its DMASW sem lane is fresh, so this carries only
            # the DVE wait (HWDGE lane reuse would add a second wait).
            nc.gpsimd.dma_start(out=out[:], in_=out_sb[:])
    nc.compile()
    return nc


_NC_CACHE = None


def _get_nc():
    global _NC_CACHE
    if _NC_CACHE is None:
        _NC_CACHE = build_nc()
    return _NC_CACHE


def _make_in_maps(x, distance, Wcf_w, Wcf_b, Wdf_w, Wdf_b, Wfc_w):
    x = np.ascontiguousarray(np.asarray(x, np.float32))
    distance = np.ascontiguousarray(np.asarray(distance, np.float32))
    x_flat = x.reshape(B * N, D)
    dist_flat = distance.reshape(B * N, N * D)
    wcfT = np.asarray(Wcf_w, np.float32).T
    wdfT = np.asarray(Wdf_w, np.float32).T
    wfcT = np.asarray(Wfc_w, np.float32).T
    bcf = np.asarray(Wcf_b, np.float32)
    bdf = np.asarray(Wdf_b, np.float32)
    in_maps = []
    for c in range(NCORES):
        sl = slice(c * ROWS, (c + 1) * ROWS)
        cstblk = np.zeros((128, C_TOT), np.float32)
        cstblk[:, C_XT:C_XT + ROWS] = x_flat[sl].T
        cstblk[:, C_WCF:C_WCF + H] = wcfT
        cstblk[:, C_WDF:C_WDF + H] = wdfT
        cstblk[:, C_WFC:C_WFC + D] = wfcT
        cstblk[:, C_EYE:C_EYE + ROWS] = np.eye(ROWS, dtype=np.float32)
        cstblk[0, C_BCFR:C_BCFR + H] = bcf
        cstblk[0, C_BDFR:C_BDFR + H] = bdf
        cstblk[0, C_ONES:C_ONES + ROWS] = 1.0
        cstblk[0, C_N256:C_N256 + ROWS] = float(N)
        in_maps.append({
            "dist": np.ascontiguousarray(dist_flat[sl]),
            "cst": cstblk,
        })
    return in_maps


def kernel(x, distance, Wcf_w, Wcf_b, Wdf_w, Wdf_b, Wfc_w):
    in_maps = _make_in_maps(x, distance, Wcf_w, Wcf_b, Wdf_w, Wdf_b, Wfc_w)
    nc = _get_nc()
    res = run_bass_kernel_spmd(nc, in_maps, list(range(NCORES))).results
    out = np.concatenate([res[c]["out"] for c in range(NCORES)], axis=0)
    return out.reshape(B, N, D)


# revision 15
# speedup vs baseline: 1.1840x; 1.1217x over previous
"""DTNN layer kernel for Trainium2 (8 NeuronCores).

Math: out[b,i,o] = sum_j sum_h Wfc[o,h] * hx[b,i,h] * hd[b,i,j,h]
with hx = x@Wcf.T + bcf, hd = dist@Wdf.T + bdf.
Since Wfc/Wdf are linear, the j-sum commutes:
    ds[b,i,d]  = sum_j dist[b,i,j,d]                  (memory-bound reduction)
    out[b,i,:] = ((x@Wcf.T + bcf) * (ds@Wdf.T + N*bdf)) @ Wfc.T
So the kernel streams `distance` once (134MB) and does a few 128x128 matmuls.

Sharding: flatten (B,N) -> 1024 i-rows, 128 rows per core; no cross-core comms.

Hardware quirks honored here:
- compute instructions tolerate only ONE sync-wait, so all constants arrive
  via a single dma_start (one sem), biases are folded into the PE matmuls as
  K=1 rank-1 updates (bias_row^T @ ones_row), and DVE ops are arranged to
  have either one cross-engine wait or same-engine deps only.
"""

import numpy as np

import concourse.bass as bass
import concourse.bacc as bacc
import concourse.mybir as mybir
from concourse.tile import TileContext
from concourse.bass_utils import run_bass_kernel_spmd

B, N, D, H = 4, 256, 128, 128
NCORES = 8
ROWS = B * N // NCORES  # 128 i-rows per core
FP = mybir.dt.float32

J_TILE = 32             # j-columns per DMA block (2MB per DMA)
N_BLK = N // J_TILE     # 8 blocks

# packed constant columns: [xT | wcfT | wdfT | wfcT | eye | rows...]
C_XT = 0
C_WCF = 128
C_WDF = 256
C_WFC = 384
C_EYE = 512
C_BCFR = 640   # partition 0: bcf row (1, H)
C_BDFR = 768   # partition 0: bdf row (1, H)
C_ONES = 896   # partition 0: ones row (1, ROWS)
C_N256 = 1024  # partition 0: value-N row (1, ROWS)
C_TOT = 1152


def build_nc():
    nc = bacc.Bacc("TRN2", target_bir_lowering=False)
    dist = nc.declare_dram_parameter("dist", [ROWS, N * D], FP, isOutput=False)
    cst = nc.declare_dram_parameter("cst", [128, C_TOT], FP, isOutput=False)
    out = nc.declare_dram_parameter("out", [ROWS, D], FP, isOutput=True)

    with TileContext(nc) as tc:
        with (
            tc.tile_pool(name="const", bufs=1) as cpool,
            tc.tile_pool(name="dist", bufs=1) as dpool,
            tc.tile_pool(name="work", bufs=1) as wpool,
            tc.tile_pool(name="psum", bufs=1, space="PSUM") as ppool,
        ):
            # Issue the dist stream first so the big DMAs start ASAP; the
            # constants ride behind them on the same queue.
            SIZES = [32] * 7 + [16, 16]  # j-counts per DMA tile (powers of 2)
            dtiles = []
            off = 0
            for k, jn in enumerate(SIZES):
                t = dpool.tile([ROWS, jn * D], FP, tag=f"dist{k}")
                nc.sync.dma_start(out=t[:], in_=dist[:, off * D:(off + jn) * D])
                dtiles.append(t)
                off += jn

            cst_t = cpool.tile([128, C_TOT], FP)
            nc.sync.dma_start(out=cst_t[:], in_=cst[:])
            xT_t = cst_t[:, C_XT:C_XT + ROWS]
            wcf_t = cst_t[:, C_WCF:C_WCF + H]
            wdf_t = cst_t[:, C_WDF:C_WDF + H]
            wfc_t = cst_t[:, C_WFC:C_WFC + D]
            ident = cst_t[:, C_EYE:C_EYE + ROWS]
            bcf_row = cst_t[0:1, C_BCFR:C_BCFR + H]
            bdf_row = cst_t[0:1, C_BDFR:C_BDFR + H]
            ones_row = cst_t[0:1, C_ONES:C_ONES + ROWS]
            n256_row = cst_t[0:1, C_N256:C_N256 + ROWS]

            # hx^T = (Wcf^T)^T @ x^T + bcf x ones -> (H, ROWS) in PSUM
            hx_ps = ppool.tile([H, ROWS], FP)
            nc.tensor.matmul(hx_ps[:], wcf_t, xT_t, start=True, stop=False)
            nc.tensor.matmul(hx_ps[:], bcf_row, ones_row, start=False, stop=True)
            hxT = wpool.tile([H, ROWS], FP)
            nc.vector.tensor_copy(hxT[:], hx_ps[:])

            # Streaming j-reduction: ds[i,d] = sum_j dist[i,j,d].
            # Each tile is folded to 128 columns in place immediately after
            # its DMA lands (halving adds, all unit-stride = full DVE rate),
            # then added into the running accumulator (tile 0). Per-tile DVE
            # work (~4.9us) keeps pace with per-tile DMA arrival (~5.1us),
            # so only ~2us of DVE work remains after the last (half-size)
            # tile arrives.
            acc = dtiles[0]
            for k, jn in enumerate(SIZES):
                t = dtiles[k]
                half = jn * D // 2
                while half >= D:
                    nc.vector.tensor_add(
                        t[:, 0:half], t[:, 0:half], t[:, half:2 * half]
                    )
                    half //= 2
                if k > 0:
                    nc.vector.tensor_add(acc[:, 0:D], acc[:, 0:D], t[:, 0:D])
            ds = acc[:, 0:D]

            # ds (i,d) -> dsT (d,i) via PE transpose
            dsT_ps = ppool.tile([D, ROWS], FP)
            nc.tensor.transpose(dsT_ps[:], ds, ident)
            dsT = wpool.tile([D, ROWS], FP)
            nc.vector.tensor_copy(dsT[:], dsT_ps[:])

            # hd^T = (Wdf^T)^T @ ds^T + N * bdf x ones -> (H, ROWS)
            hd_ps = ppool.tile([H, ROWS], FP)
            nc.tensor.matmul(hd_ps[:], wdf_t, dsT[:], start=True, stop=False)
            nc.tensor.matmul(hd_ps[:], bdf_row, n256_row, start=False, stop=True)

            # s^T = hx^T * hd^T (read PSUM directly)
            sT = wpool.tile([H, ROWS], FP)
            nc.vector.tensor_mul(sT[:], hd_ps[:], hxT[:])

            # out = sT^T @ Wfc^T -> (ROWS, D)
            out_ps = ppool.tile([ROWS, D], FP)
            nc.tensor.matmul(out_ps[:], sT[:], wfc_t, start=True, stop=True)
            out_sb = wpool.tile([ROWS, D], FP)
            nc.vector.tensor_copy(out_sb[:], out_ps[:])
            # SWDGE store: its DMASW sem lane is fresh, so this carries only
            # the DVE wait (HWDGE lane reuse would add a second wait).
            nc.gpsimd.dma_start(out=out[:], in_=out_sb[:])
    nc.compile()
    return nc


_NC_CACHE = None


def _get_nc():
    global _NC_CACHE
    if _NC_CACHE is None:
        _NC_CACHE = build_nc()
    return _NC_CACHE


def _make_in_maps(x, distance, Wcf_w, Wcf_b, Wdf_w, Wdf_b, Wfc_w):
    x = np.ascontiguousarray(np.asarray(x, np.float32))
    distance = np.ascontiguousarray(np.asarray(distance, np.float32))
    x_flat = x.reshape(B * N, D)
    dist_flat = distance.reshape(B * N, N * D)
    wcfT = np.asarray(Wcf_w, np.float32).T
    wdfT = np.asarray(Wdf_w, np.float32).T
    wfcT = np.asarray(Wfc_w, np.float32).T
    bcf = np.asarray(Wcf_b, np.float32)
    bdf = np.asarray(Wdf_b, np.float32)
    in_maps = []
    for c in range(NCORES):
        sl = slice(c * ROWS, (c + 1) * ROWS)
        cstblk = np.zeros((128, C_TOT), np.float32)
        cstblk[:, C_XT:C_XT + ROWS] = x_flat[sl].T
        cstblk[:, C_WCF:C_WCF + H] = wcfT
        cstblk[:, C_WDF:C_WDF + H] = wdfT
        cstblk[:, C_WFC:C_WFC + D] = wfcT
        cstblk[:, C_EYE:C_EYE + ROWS] = np.eye(ROWS, dtype=np.float32)
        cstblk[0, C_BCFR:C_BCFR + H] = bcf
        cstblk[0, C_BDFR:C_BDFR + H] = bdf
        cstblk[0, C_ONES:C_ONES + ROWS] = 1.0
        cstblk[0, C_N256:C_N256 + ROWS] = float(N)
        in_maps.append({
            "dist": np.ascontiguousarray(dist_flat[sl]),
            "cst": cstblk,
        })
    return in_maps


def kernel(x, distance, Wcf_w, Wcf_b, Wdf_w, Wdf_b, Wfc_w):
    in_maps = _make_in_maps(x, distance, Wcf_w, Wcf_b, Wdf_w, Wdf_b, Wfc_w)
    nc = _get_nc()
    res = run_bass_kernel_spmd(nc, in_maps, list(range(NCORES))).results
    out = np.concatenate([res[c]["out"] for c in range(NCORES)], axis=0)
    return out.reshape(B, N, D)
